# revision 2
# baseline (speedup 1.0000x reference)
"""Transformer decoder block (self-attn + cross-attn + FFN, post-LN) on 8
Trainium2 NeuronCores.

Sharding: zero-communication data parallel. 8 cores = 2 batches x 4
query-chunks of 512 tokens. Each core redundantly computes K/V projections
for its batch's full 2048 tokens (K/V sources are raw kernel inputs, so no
collective is needed), runs attention for its 512 queries over all keys,
and the FFN/LayerNorms for its own tokens. Host splits inputs / concats
outputs.

On-chip layouts:
  - Projections produce Q^T/K^T as [feature, token] (feature on partitions)
    so the scores matmul contracts dh on partitions with 2-head row packing.
  - V is produced as [token, dh] so AV contracts keys on partitions with
    2-head column packing.
  - Scores are computed transposed (S^T = [key, query]); exp runs on the
    scalar engine straight out of PSUM; the causal mask is a multiplicative
    bf16 input applied on the vector engine.
  - Softmax denominators: P^T tiles are accumulated over key-tiles on the
    vector engine, an M=1 matmul sums partitions, one reciprocal, then a
    K=1 matmul broadcasts 1/Z across partitions for the normalize multiply.
  - Residual + LayerNorm run in [token, feature] layout (bn_stats/bn_aggr),
    then a PE transpose produces the [feature, token] operand for the next
    block's projections.
All matmuls bf16 with fp32 PSUM accumulation; residual/LN paths fp32.
"""

from contextlib import ExitStack

import numpy as np
import ml_dtypes

import concourse.bass as bass
import concourse.bacc as bacc
import concourse.mybir as mybir
import concourse.tile as tile
from concourse import bass_utils
from concourse.masks import make_identity

BF16 = mybir.dt.bfloat16
F32 = mybir.dt.float32
AF = mybir.ActivationFunctionType
OP = mybir.AluOpType

B, S, D, H, F = 2, 2048, 1024, 16, 4096
DH = 64
EPS = 1e-5
CH = 512          # tokens per core
DT = D // 128     # 8 feature tiles
NKT = S // 128    # 16 key tiles
NPAIR = H // 2    # 8 head pairs
NMT = CH // 128   # 4 token tiles per core
NFT = F // 128    # 32 FFN hidden tiles

_CACHED = None


def build():
    nc = bacc.Bacc("TRN2", target_bir_lowering=False, debug=False,
                   enable_asserts=False, num_devices=8)

    # ---- per-core DRAM I/O ----
    d_xTq = nc.dram_tensor("xTq", [D, CH], BF16, kind="ExternalInput")
    d_eTq = nc.dram_tensor("eTq", [D, CH], BF16, kind="ExternalInput")
    d_res1 = nc.dram_tensor("res1", [CH, D], F32, kind="ExternalInput")
    d_maskT = nc.dram_tensor("maskT", [S, CH], BF16, kind="ExternalInput")
    wnames = ["sa_wq", "sa_wk", "sa_wv", "sa_wo", "ca_wq", "ca_wk", "ca_wv", "ca_wo"]
    d_w = {n: nc.dram_tensor(n, [D, D], BF16, kind="ExternalInput") for n in wnames}
    d_w1 = nc.dram_tensor("f_w1", [D, F], BF16, kind="ExternalInput")
    d_w2 = nc.dram_tensor("f_w2", [F, D], BF16, kind="ExternalInput")
    d_bq_sa = nc.dram_tensor("sa_bq", [D], F32, kind="ExternalInput")
    d_bk_sa = nc.dram_tensor("sa_bk", [D], F32, kind="ExternalInput")
    d_bq_ca = nc.dram_tensor("ca_bq", [D], F32, kind="ExternalInput")
    d_bk_ca = nc.dram_tensor("ca_bk", [D], F32, kind="ExternalInput")
    d_b1 = nc.dram_tensor("f_b1", [F], F32, kind="ExternalInput")
    d_cvec = nc.dram_tensor("cvec", [D], BF16, kind="ExternalInput")
    d_b2v = nc.dram_tensor("b2v", [D], BF16, kind="ExternalInput")
    d_gbt = {n: nc.dram_tensor(n, [D], BF16, kind="ExternalInput")
             for n in ["sa_g", "sa_bt", "ca_g", "ca_bt", "f_g", "f_bt"]}
    d_out = nc.dram_tensor("out", [CH, D], F32, kind="ExternalOutput")
    cc = {}
    for pfx in ("sa", "ca"):
        cc[f"{pfx}_kt_in"] = nc.dram_tensor(f"cc_{pfx}_kt_in", [D, CH], BF16,
                                            kind="Internal")
        cc[f"{pfx}_kt_out"] = nc.dram_tensor(f"cc_{pfx}_kt_out", [4 * D, CH],
                                             BF16, kind="Internal")
        cc[f"{pfx}_v_in"] = nc.dram_tensor(f"cc_{pfx}_v_in", [CH, D], BF16,
                                           kind="Internal")
        cc[f"{pfx}_v_out"] = nc.dram_tensor(f"cc_{pfx}_v_out", [S, D], BF16,
                                            kind="Internal")
    GROUPS = [[0, 1, 2, 3], [4, 5, 6, 7]]

    with tile.TileContext(nc) as tc, ExitStack() as ctx:
        const = ctx.enter_context(tc.tile_pool(name="const", bufs=1))
        wpool = ctx.enter_context(tc.tile_pool(name="wpool", bufs=8))
        qpool = ctx.enter_context(tc.tile_pool(name="qpool", bufs=16))
        resp = ctx.enter_context(tc.tile_pool(name="resp", bufs=8))
        scrp = ctx.enter_context(tc.tile_pool(name="scrp", bufs=2))
        ps_s = ctx.enter_context(tc.tile_pool(name="ps_s", bufs=2, space="PSUM"))
        ps_av = ctx.enter_context(tc.tile_pool(name="ps_av", bufs=2, space="PSUM"))
        ps_m = ctx.enter_context(tc.tile_pool(name="ps_m", bufs=3, space="PSUM"))

        ident = const.tile([128, 128], F32, tag="ident")
        make_identity(nc, ident)
        onescol = const.tile([128, 1], BF16, tag="onescol")
        nc.vector.memset(onescol, 1.0)
        onesrow = const.tile([1, 64], F32, tag="onesrow")
        nc.vector.memset(onesrow, 1.0)
        epst = const.tile([128, 1], F32, tag="epst")
        nc.vector.memset(epst, EPS)
        zerot = const.tile([128, 1], F32, tag="zerot")
        nc.vector.memset(zerot, 0.0)

        def bias_cols(dram, ntiles, name):
            t = const.tile([128, ntiles], F32, tag=name, name=name)
            src = bass.AP(tensor=dram.ap().tensor, offset=0,
                          ap=[[1, 128], [128, ntiles]])
            nc.sync.dma_start(out=t, in_=src)
            return t

        def bcast_row(dram, tag, name):
            t = const.tile([128, D], BF16, tag=tag, bufs=2, name=name)
            src = bass.AP(tensor=dram.ap().tensor, offset=0, ap=[[0, 128], [1, D]])
            nc.sync.dma_start(out=t, in_=src)
            return t

        bq_sa = bias_cols(d_bq_sa, DT, "bqsa")
        bk_sa = bias_cols(d_bk_sa, DT, "bksa")
        bq_ca = bias_cols(d_bq_ca, DT, "bqca")
        bk_ca = bias_cols(d_bk_ca, DT, "bkca")
        b1c = bias_cols(d_b1, NFT, "b1c")

        def layer_norm(src, g_t, bt_t, out):
            """[128, D] f32 LN along free dim; out may alias src."""
            stats = scrp.tile([128, 2, 6], F32, tag="lnstat", name="lnstat")
            for s in range(2):
                nc.vector.bn_stats(out=stats[:, s, :],
                                   in_=src[:, s * 512:(s + 1) * 512])
            mv = scrp.tile([128, 2], F32, tag="lnmv", name="lnmv")
            nc.vector.bn_aggr(out=mv, in_=stats)
            rstd = scrp.tile([128, 1], F32, tag="lnrstd", name="lnrstd")
            nc.scalar.activation(out=rstd, in_=mv[:, 1:2], func=AF.Sqrt,
                                 bias=epst, scale=1.0)
            nc.vector.reciprocal(out=rstd, in_=rstd)
            cent = scrp.tile([128, D], F32, tag="scr", name="cent")
            nc.vector.scalar_tensor_tensor(out=cent, in0=src, scalar=mv[:, 0:1],
                                           in1=g_t, op0=OP.subtract, op1=OP.mult)
            nc.vector.scalar_tensor_tensor(out=out, in0=cent, scalar=rstd,
                                           in1=bt_t, op0=OP.mult, op1=OP.add)

        def load_w8(wd, ncols=D):
            ws = []
            for k in range(DT):
                t = wpool.tile([128, ncols], BF16, tag="w", name=f"w_{k}")
                nc.sync.dma_start(out=t, in_=wd.ap()[k * 128:(k + 1) * 128, :])
                ws.append(t)
            return ws

        def projT(ws, src_tiles, bias_col, out_tag):
            """out^T [feature, token] tiles: lhsT=weight cols, rhs=src^T."""
            outs = []
            for m in range(DT):
                ps = ps_m.tile([128, CH], F32, tag="ps_m", name="projps")
                for k in range(DT):
                    nc.tensor.matmul(ps, ws[k][:, m * 128:(m + 1) * 128],
                                     src_tiles[k], start=(k == 0),
                                     stop=(k == DT - 1))
                o = qpool.tile([128, CH], BF16, tag="qt", name=f"{out_tag}{m}")
                nc.scalar.activation(out=o, in_=ps, func=AF.Identity,
                                     bias=bias_col[:, m:m + 1], scale=1.0)
                outs.append(o)
            return outs

        def attention(pfx, d_ktout, d_vout, QT, masks, pools):
            attp, kvp, vpp, ppool, zpool, zsm = pools
            zall = zsm.tile([16, CH], F32, tag="zall", name="zall")
            aun = []
            vcur = None
            for hp in range(NPAIR):
                with nc.named_scope(f"{pfx}_pair{hp}"):
                    # K^T for this head pair from the AllGather buffer:
                    # [128 (2 heads x 64 dh), S], chunk c at rows 1024c+128hp
                    ktp = kvp.tile([128, 4, CH], BF16, tag="ktp", name="ktp")
                    nc.sync.dma_start(
                        out=ktp,
                        in_=bass.AP(tensor=d_ktout.ap().tensor,
                                    offset=128 * hp * CH,
                                    ap=[[CH, 128], [D * CH, 4], [1, CH]]))
                    ktp = ktp.rearrange("p a q -> p (a q)")
                    # V for pair-pair from the AllGather buffer
                    if hp % 2 == 0:
                        vt = vpp.tile([128, NKT, 256], BF16, tag="vpp", bufs=1,
                                      name="vpp")
                        nc.sync.dma_start(
                            out=vt,
                            in_=bass.AP(tensor=d_vout.ap().tensor,
                                        offset=(hp // 2) * 256,
                                        ap=[[D, 128], [128 * D, NKT], [1, 256]]))
                        vcur = vt.rearrange("p a q -> p (a q)")
                    voff = (hp % 2) * 128

                    qa = QT[hp][0:64, :]
                    qb = QT[hp][64:128, :]
                    pav = ps_av.tile([128, CH], F32, tag="ps_av", bufs=1,
                                     name="pav")
                    zacc = zpool.tile([128, 4 * CH], BF16, tag="zacc", bufs=1,
                                      name="zacc")
                    for kt2 in range(NKT // 2):
                        pt2 = ppool.tile([128, 4 * CH], BF16, tag="pt", name="pt")
                        for sub in range(2):
                            kt = 2 * kt2 + sub
                            pss = ps_s.tile([128, 2 * CH], F32, tag="ps_s",
                                            name="pss")
                            ksl = ktp[:, kt * 128:(kt + 1) * 128]
                            nc.tensor.matmul(pss[:, 0:CH], ksl[0:64, :], qa,
                                             start=True, stop=True)
                            nc.tensor.matmul(pss[:, CH:2 * CH], ksl[64:128, :],
                                             qb, start=True, stop=True)
                            nc.scalar.activation(
                                out=pt2[:, sub * 2 * CH:(sub + 1) * 2 * CH],
                                in_=pss, func=AF.Exp, bias=zerot,
                                scale=1.0 / np.sqrt(DH))
                        if masks is not None:
                            mk = masks(kt2)
                            ptv = pt2.rearrange("p (a q) -> p a q", a=4)
                            nc.vector.tensor_mul(
                                ptv[:, 0::2, :], ptv[:, 0::2, :], mk)
                            nc.vector.tensor_mul(
                                ptv[:, 1::2, :], ptv[:, 1::2, :], mk)
                        if kt2 == 0:
                            nc.vector.tensor_copy(zacc, pt2)
                        else:
                            nc.vector.tensor_add(zacc, zacc, pt2)
                        for sub in range(2):
                            kt = 2 * kt2 + sub
                            po = sub * 2 * CH
                            vsl = vcur[:, kt * 256 + voff: kt * 256 + voff + 128]
                            nc.tensor.matmul(pav[0:64, :], vsl[:, 0:64],
                                             pt2[:, po:po + CH],
                                             start=(kt == 0),
                                             stop=(kt == NKT - 1))
                            nc.tensor.matmul(pav[64:128, :], vsl[:, 64:128],
                                             pt2[:, po + CH:po + 2 * CH],
                                             start=(kt == 0),
                                             stop=(kt == NKT - 1))
                    at = attp.tile([128, CH], BF16, tag="aun", name=f"aun{hp}")
                    nc.vector.tensor_copy(at, pav)
                    aun.append(at)
                    # Z rows: partition-sum of zacc via M=1 matmuls
                    # zacc layout: [ktA-hA | ktA-hB | ktB-hA | ktB-hB] x 512
                    for h2 in range(2):
                        zf = ps_m.tile([1, CH], F32, tag="ps_m", name="zf")
                        nc.tensor.matmul(zf, onescol,
                                         zacc[:, h2 * CH:(h2 + 1) * CH],
                                         start=True, stop=False)
                        nc.tensor.matmul(zf, onescol,
                                         zacc[:, 2 * CH + h2 * CH:
                                              2 * CH + (h2 + 1) * CH],
                                         start=False, stop=True)
                        zrow = zsm.tile([1, CH], F32, tag="zrow", bufs=2,
                                        name="zrow")
                        nc.vector.tensor_copy(zrow, zf)
                        nc.sync.dma_start(
                            out=zall[2 * hp + h2:2 * hp + h2 + 1, :], in_=zrow)
            # normalize: rz = 1/Z broadcast over the 64 dh partitions per head
            rz = zsm.tile([16, CH], F32, tag="rz", name="rz")
            nc.vector.reciprocal(out=rz, in_=zall)
            for hp in range(NPAIR):
                # matmul moving operands need partition base 0 -> DMA-scatter
                rza = zsm.tile([1, CH], F32, tag="rzrow", bufs=4, name="rza")
                rzb = zsm.tile([1, CH], F32, tag="rzrow", bufs=4, name="rzb")
                nc.sync.dma_start(out=rza, in_=rz[2 * hp:2 * hp + 1, :])
                nc.sync.dma_start(out=rzb, in_=rz[2 * hp + 1:2 * hp + 2, :])
                prz = ps_m.tile([128, CH], F32, tag="ps_m", name="prz")
                nc.tensor.matmul(prz[0:64, :], onesrow[0:1, :], rza,
                                 start=True, stop=True, tile_position=(0, 0))
                nc.tensor.matmul(prz[64:128, :], onesrow[0:1, :], rzb,
                                 start=True, stop=True, tile_position=(0, 64))
                nc.vector.tensor_mul(aun[hp], aun[hp], prz)
            return aun

        def kv_local_and_ag(pfx, d_wk, d_wv, bk_col, src_tiles):
            """Project this chunk's K^T/V, stage to DRAM, AllGather per batch."""
            with nc.named_scope(f"{pfx}_kvlocal"):
                wk = load_w8(d_wk)
                for m in range(DT):
                    ps = ps_m.tile([128, CH], F32, tag="ps_m", name="lkps")
                    for k in range(DT):
                        nc.tensor.matmul(ps, wk[k][:, m * 128:(m + 1) * 128],
                                         src_tiles[k], start=(k == 0),
                                         stop=(k == DT - 1))
                    st = scrp.tile([128, CH], BF16, tag="stage", bufs=4,
                                   name="ktst")
                    nc.scalar.activation(out=st, in_=ps, func=AF.Identity,
                                         bias=bk_col[:, m:m + 1], scale=1.0)
                    nc.sync.dma_start(
                        out=cc[f"{pfx}_kt_in"].ap()[m * 128:(m + 1) * 128, :],
                        in_=st)
                wv = load_w8(d_wv)
                for tt in range(NMT):
                    for n in range(2):
                        ps = ps_m.tile([128, CH], F32, tag="ps_m", name="lvps")
                        for k in range(DT):
                            nc.tensor.matmul(
                                ps, src_tiles[k][:, tt * 128:(tt + 1) * 128],
                                wv[k][:, n * 512:(n + 1) * 512],
                                start=(k == 0), stop=(k == DT - 1))
                        st = scrp.tile([128, CH], BF16, tag="stage", bufs=4,
                                       name="vst")
                        nc.scalar.activation(out=st, in_=ps, func=AF.Copy)
                        nc.sync.dma_start(
                            out=cc[f"{pfx}_v_in"].ap()[tt * 128:(tt + 1) * 128,
                                                       n * 512:(n + 1) * 512],
                            in_=st)
                nc.gpsimd.collective_compute(
                    "AllGather", mybir.AluOpType.bypass,
                    ins=[cc[f"{pfx}_kt_in"].ap()],
                    outs=[cc[f"{pfx}_kt_out"].ap()],
                    replica_groups=GROUPS)
                nc.gpsimd.collective_compute(
                    "AllGather", mybir.AluOpType.bypass,
                    ins=[cc[f"{pfx}_v_in"].ap()],
                    outs=[cc[f"{pfx}_v_out"].ap()],
                    replica_groups=GROUPS)

        def wo_resid_ln(attnT, d_wo, resid_fn, extra_vec, g_t, bt_t, tag):
            """WO matmul + residual + LN in [token, feature]; in-place LN."""
            wo = load_w8(d_wo)
            outs = []
            for mt in range(NMT):
                pre = resp.tile([128, D], F32, tag="persist", name=f"{tag}{mt}")
                rt = resid_fn(mt)
                for n in range(2):
                    ps = ps_m.tile([128, 512], F32, tag="ps_m", name="wops")
                    for k in range(DT):
                        nc.tensor.matmul(
                            ps, attnT[k][:, mt * 128:(mt + 1) * 128],
                            wo[k][:, n * 512:(n + 1) * 512],
                            start=(k == 0), stop=(k == DT - 1))
                    nc.vector.tensor_add(pre[:, n * 512:(n + 1) * 512], ps,
                                         rt[:, n * 512:(n + 1) * 512])
                if extra_vec is not None:
                    nc.vector.tensor_add(pre, pre, extra_vec)
                layer_norm(pre, g_t, bt_t, pre)
                outs.append(pre)
            return outs

        def transposeT(x_tiles, out_tag):
            """4 [128, D] f32 token-major -> 8 [128, CH] bf16 feature-major."""
            outs = [qpool.tile([128, CH], BF16, tag="qt",
                               name=f"{out_tag}{i}") for i in range(DT)]
            for mt in range(NMT):
                for ft in range(DT):
                    pst = ps_m.tile([128, 128], F32, tag="ps_m", name="tps")
                    nc.tensor.transpose(
                        pst, x_tiles[mt][:, ft * 128:(ft + 1) * 128], ident)
                    nc.vector.tensor_copy(
                        outs[ft][:, mt * 128:(mt + 1) * 128], pst)
            return outs

        # ======== attention phases (pools released before FFN) ========
        with ExitStack() as attn_ctx:
            maskp = attn_ctx.enter_context(tc.tile_pool(name="maskp", bufs=2))
            kvp = attn_ctx.enter_context(tc.tile_pool(name="kvp", bufs=2))
            vpp = attn_ctx.enter_context(tc.tile_pool(name="vpp", bufs=1))
            ppool = attn_ctx.enter_context(tc.tile_pool(name="ppool", bufs=3))
            zpool = attn_ctx.enter_context(tc.tile_pool(name="zpool", bufs=1))
            attp = attn_ctx.enter_context(tc.tile_pool(name="attp", bufs=8))
            zsm = attn_ctx.enter_context(tc.tile_pool(name="zsm", bufs=1))
            pools = (attp, kvp, vpp, ppool, zpool, zsm)

            xq = []
            for k in range(DT):
                t = qpool.tile([128, CH], BF16, tag="qt", name=f"xq{k}")
                nc.sync.dma_start(out=t, in_=d_xTq.ap()[k * 128:(k + 1) * 128, :])
                xq.append(t)
            mbs = []
            for b in range(2):
                mb = maskp.tile([128, 8 * CH], BF16, tag="mask", bufs=2,
                                name=f"maskb{b}")
                nc.sync.dma_start(
                    out=mb,
                    in_=bass.AP(tensor=d_maskT.ap().tensor,
                                offset=b * 8 * 128 * CH,
                                ap=[[CH, 128], [128 * CH, 8], [1, CH]]))
                mbs.append(mb)

            def masks(kt2):
                # [128, 2, CH] view covering key tiles 2*kt2, 2*kt2+1
                b, i = (2 * kt2) // 8, (2 * kt2) % 8
                return mbs[b][:, i * CH:(i + 2) * CH].rearrange(
                    "p (a q) -> p a q", a=2)
            # local K/V + AllGather for both attentions, issued up front
            # so the collectives overlap with Q projection / SA attention
            kv_local_and_ag("sa", d_w["sa_wk"], d_w["sa_wv"], bk_sa, xq)
            eq = []
            for k in range(DT):
                t = qpool.tile([128, CH], BF16, tag="qt", name=f"eq{k}")
                nc.sync.dma_start(out=t, in_=d_eTq.ap()[k * 128:(k + 1) * 128, :])
                eq.append(t)
            kv_local_and_ag("ca", d_w["ca_wk"], d_w["ca_wv"], bk_ca, eq)

            # ---- self attention ----
            with nc.named_scope("sa_q"):
                wq = load_w8(d_w["sa_wq"])
                QTsa = projT(wq, xq, bq_sa, "qsa")
            attnT = attention("sa", cc["sa_kt_out"], cc["sa_v_out"],
                              QTsa, masks, pools)

            def sa_resid(mt):
                t = scrp.tile([128, D], F32, tag="res1", name="res1t")
                nc.sync.dma_start(
                    out=t, in_=d_res1.ap()[mt * 128:(mt + 1) * 128, :])
                return t

            with nc.named_scope("sa_wo_ln"):
                g1 = bcast_row(d_gbt["sa_g"], "gt", "g1")
                bt1 = bcast_row(d_gbt["sa_bt"], "gt", "bt1")
                x1 = wo_resid_ln(attnT, d_w["sa_wo"], sa_resid, None,
                                 g1, bt1, "x1_")
                x1T = transposeT(x1, "x1T")

            # ---- cross attention ----
            with nc.named_scope("ca_q"):
                wqc = load_w8(d_w["ca_wq"])
                QTca = projT(wqc, x1T, bq_ca, "qca")
            attnTc = attention("ca", cc["ca_kt_out"], cc["ca_v_out"],
                               QTca, None, pools)
            with nc.named_scope("ca_wo_ln"):
                cvec_t = bcast_row(d_cvec, "vec", "cvec")
                g2 = bcast_row(d_gbt["ca_g"], "gt", "g2")
                bt2 = bcast_row(d_gbt["ca_bt"], "gt", "bt2")
                y1 = wo_resid_ln(attnTc, d_w["ca_wo"],
                                 lambda mt: x1[mt],
                                 cvec_t, g2, bt2, "y1_")
                y1T = transposeT(y1, "y1T")

        # ======== FFN ========
        with ExitStack() as ffn_ctx:
            hpool = ffn_ctx.enter_context(tc.tile_pool(name="hpool", bufs=32))
            w1pool = ffn_ctx.enter_context(tc.tile_pool(name="w1pool", bufs=8))
            with nc.named_scope("ffn1"):
                w1 = []
                for k in range(DT):
                    t = w1pool.tile([128, F], BF16, tag="w1", name=f"w1_{k}")
                    nc.sync.dma_start(out=t,
                                      in_=d_w1.ap()[k * 128:(k + 1) * 128, :])
                    w1.append(t)
                hT = []
                for m in range(NFT):
                    ps = ps_m.tile([128, CH], F32, tag="ps_m", name="f1ps")
                    for k in range(DT):
                        nc.tensor.matmul(ps, w1[k][:, m * 128:(m + 1) * 128],
                                         y1T[k], start=(k == 0),
                                         stop=(k == DT - 1))
                    h = hpool.tile([128, CH], BF16, tag="h", name=f"h{m}")
                    nc.scalar.activation(out=h, in_=ps, func=AF.Relu,
                                         bias=b1c[:, m:m + 1], scale=1.0)
                    hT.append(h)
            with nc.named_scope("ffn2"):
                b2v_t = bcast_row(d_b2v, "vec", "b2v")
                h2 = [resp.tile([128, D], F32, tag="persist", name=f"h2_{i}")
                      for i in range(NMT)]
                for n in range(2):
                    pss = [ps_s.tile([128, 2 * CH], F32, tag="ps_s",
                                     name=f"f2ps{n}_{i}") for i in range(2)]
                    for kb in range(4):
                        w2b = w1pool.tile([128, 8, 512], BF16, tag="w1",
                                          name=f"w2b{kb}")
                        nc.sync.dma_start(
                            out=w2b,
                            in_=bass.AP(tensor=d_w2.ap().tensor,
                                        offset=kb * 8 * 128 * D + n * 512,
                                        ap=[[D, 128], [128 * D, 8], [1, 512]]))
                        for ks in range(8):
                            k = kb * 8 + ks
                            for mt in range(NMT):
                                nc.tensor.matmul(
                                    pss[mt // 2][:, (mt % 2) * CH:
                                                 (mt % 2 + 1) * CH],
                                    hT[k][:, mt * 128:(mt + 1) * 128],
                                    w2b[:, ks, :],
                                    start=(k == 0), stop=(k == NFT - 1))
                    for mt in range(NMT):
                        nc.vector.tensor_add(
                            h2[mt][:, n * 512:(n + 1) * 512],
                            pss[mt // 2][:, (mt % 2) * CH:(mt % 2 + 1) * CH],
                            y1[mt][:, n * 512:(n + 1) * 512])
            with nc.named_scope("ln3_out"):
                g3 = bcast_row(d_gbt["f_g"], "gt", "g3")
                bt3 = bcast_row(d_gbt["f_bt"], "gt", "bt3")
                for mt in range(NMT):
                    nc.vector.tensor_add(h2[mt], h2[mt], b2v_t)
                    layer_norm(h2[mt], g3, bt3, h2[mt])
                    nc.sync.dma_start(out=d_out.ap()[mt * 128:(mt + 1) * 128, :],
                                      in_=h2[mt])

    nc.compile()
    return nc


def _bf(a):
    return np.ascontiguousarray(a, dtype=np.float32).astype(ml_dtypes.bfloat16)


def kernel(**inputs):
    global _CACHED
    if _CACHED is None:
        _CACHED = build()
    nc = _CACHED

    f = {k: np.asarray(v, dtype=np.float32) for k, v in inputs.items()}
    dec, enc = f["decoder_input"], f["encoder_output"]
    cvec = (f["ca_bv"] @ f["ca_wo"] + f["ca_bo"]).astype(np.float32)
    r1vec = (f["sa_bv"] @ f["sa_wo"] + f["sa_bo"]).astype(np.float32)

    shared = {n: _bf(f[n]) for n in
              ["sa_wq", "sa_wk", "sa_wv", "sa_wo",
               "ca_wq", "ca_wk", "ca_wv", "ca_wo", "f_w1", "f_w2"]}
    shared.update({n: f[n] for n in ["sa_bq", "sa_bk", "ca_bq", "ca_bk", "f_b1"]})
    shared["cvec"] = _bf(cvec)
    shared["b2v"] = _bf(f["f_b2"])
    for n in ["sa_g", "sa_bt", "ca_g", "ca_bt", "f_g", "f_bt"]:
        shared[n] = _bf(f[n])

    kk = np.arange(S, dtype=np.int64)[:, None]
    in_maps = []
    for c in range(8):
        b, j = c // 4, c % 4
        rows = slice(j * CH, (j + 1) * CH)
        qq = np.arange(j * CH, (j + 1) * CH, dtype=np.int64)[None, :]
        m = {
            "xTq": _bf(dec[b, rows, :].T),
            "eTq": _bf(enc[b, rows, :].T),
            "res1": np.ascontiguousarray(dec[b, rows, :] + r1vec[None, :]),
            "maskT": (kk <= qq).astype(ml_dtypes.bfloat16),
        }
        m.update(shared)
        in_maps.append(m)

    global LAST_RES
    res = bass_utils.run_bass_kernel_spmd(nc, in_maps, core_ids=list(range(8)))
    LAST_RES = res
    out = np.empty((B, S, D), dtype=np.float32)
    for c in range(8):
        b, j = c // 4, c % 4
        out[b, j * CH:(j + 1) * CH, :] = res.results[c]["out"]
    return out

